# revision 1
# baseline (speedup 1.0000x reference)
"""Trainium2 Bass kernel: 15x15 valid cross-correlation of 4096x4096 (+bias).

Sharding: output columns split across 8 NeuronCores (512 cols/core; each core
gets a [4096, 526] column slab of x — 14-col halo), gathered on the host.

Per core the conv runs as banded-Toeplitz matmuls on the TensorEngine:
for each of 36 row-blocks (M=114 out rows, K=M+14=128 input rows on the
partition dim) accumulate 15 matmuls into one PSUM bank —

    psum[:M, :512] += T_kw.T @ x_blk[:, kw : kw+512]      kw = 0..14

where T_kw[h_in, h_out] = wt[h_in - h_out, kw] holds the kh taps as a band
and the kw tap is a free-dim offset into the same SBUF tile (no data
duplication). The 15 Toeplitz matrices are built host-side, cast to bf16
(matmuls accumulate fp32 in PSUM; measured rel err ~2e-3); the bias rides
the PSUM->SBUF drain on the VectorEngine. Measured ~137 us on hardware:
TensorE streams 540 back-to-back matmuls at ~216 ns (N=512 @ 2.4 GHz) with
zero gaps; the remainder is NEFF preamble + the Tile exit barrier.
"""

import numpy as np

H = 4096
W = 4096
KH = 15
KW = 15
OH = H - KH + 1  # 4082
OW = W - KW + 1  # 4082
NCORES = 8
COLS = 512              # output cols per core
INC = COLS + KW - 1     # 526
BLK = 114               # output rows per row-block
NBLK = (OH + BLK - 1) // BLK  # 36 (last block M=92)

_CACHE = {}


def _build_program():
    import concourse.tile as tile
    from concourse import bacc, mybir
    from contextlib import ExitStack

    nc = bacc.Bacc("TRN2", target_bir_lowering=False, debug=False,
                   num_devices=NCORES)
    bf16 = mybir.dt.bfloat16
    f32 = mybir.dt.float32
    x_d = nc.dram_tensor("x", [H, INC], bf16, kind="ExternalInput").ap()
    w_d = nc.dram_tensor("wt", [128, KW * BLK], bf16,
                         kind="ExternalInput").ap()
    b_d = nc.dram_tensor("bias", [128, 1], f32, kind="ExternalInput").ap()
    o_d = nc.dram_tensor("out", [OH, COLS], bf16, kind="ExternalOutput").ap()

    with ExitStack() as ctx:
        tc = ctx.enter_context(tile.TileContext(nc))
        wpool = ctx.enter_context(tc.tile_pool(name="wp", bufs=1))
        bpool = ctx.enter_context(tc.tile_pool(name="bp", bufs=1))
        xpool = ctx.enter_context(tc.tile_pool(name="xp", bufs=3))
        opool = ctx.enter_context(tc.tile_pool(name="op", bufs=3))
        pspool = ctx.enter_context(tc.tile_pool(name="ps", bufs=2, space="PSUM"))
        pslast = ctx.enter_context(tc.tile_pool(name="pl", bufs=1, space="PSUM"))

        # warm the HAM clock gate during the DMA-bound startup window: dummy
        # matmuls on a zeroed scratch tile bridge from the end of the entry
        # barrier to the weight/x0 DMA landing, so the real stream starts at
        # 2.4 GHz (gpsimd memset dispatches earlier than vector's here)
        # HAM ramp: a [128,1] memset lands ~6.3us (vs ~6.9 for a wide one);
        # N=1 matmuls at the NX floor (~50ns) accumulate PE-busy time so the
        # clock gate opens ~1us before the first real matmul needs it
        z1 = wpool.tile([128, 1], bf16, tag="z1")
        nc.gpsimd.memset(z1[:], 0.0)
        wps = pspool.tile([8, 256], f32, tag="warm")
        for _ in range(40):
            nc.tensor.matmul(wps[:1, :1], z1[:, :1], z1[:, :1],
                             start=True, stop=True, skip_group_check=True)
        scr = wpool.tile([128, 256], bf16, tag="scr")
        nc.gpsimd.memset(scr[:], 0.0)
        for _ in range(12):
            nc.tensor.matmul(wps[:, :], scr[:, :8], scr[:, :],
                             start=True, stop=True, skip_group_check=True)

        # bias first (it gates the first drain); weights land in 4 chunks,
        # need-ordered, so block 0's MM #kw never waits on later taps
        b_t = bpool.tile([128, 1], f32)
        nc.gpsimd.dma_start(b_t[:], b_d[:])
        wt_t = wpool.tile([128, KW * BLK], bf16)
        for eng, k0, k1 in ((nc.scalar, 0, 2), (nc.gpsimd, 2, 4),
                            (nc.scalar, 4, 7), (nc.gpsimd, 7, 10)):
            eng.dma_start(wt_t[:, k0 * BLK: k1 * BLK],
                          w_d[:, k0 * BLK: k1 * BLK])

        # block 0 runs as two column halves so the first matmul only waits
        # for a 69KB transfer; the h1 half and the kw10-14 taps aren't
        # needed until ~3us later, so they queue behind x0a on sync
        HW2 = COLS // 2 + KW - 1  # 270
        x_hs = []
        for h in range(2):
            x_h = xpool.tile([128, HW2], bf16, tag=f"x0{h}")
            eng = nc.sync if h == 0 else nc.scalar
            eng.dma_start(x_h[:, :], x_d[0:128, h * (COLS // 2): h * (COLS // 2) + HW2])
            if h == 0:
                nc.sync.dma_start(wt_t[:, 10 * BLK: KW * BLK],
                                  w_d[:, 10 * BLK: KW * BLK])
            x_hs.append(x_h)
        for h in range(2):
            x_h = x_hs[h]
            ps = pslast.tile([BLK, COLS // 2], f32, tag=f"pl{h}")
            for kw in range(KW):
                nc.tensor.matmul(
                    ps[:BLK, :],
                    wt_t[:128, kw * BLK: kw * BLK + BLK],
                    x_h[:, kw: kw + COLS // 2],
                    start=(kw == 0),
                    stop=(kw == KW - 1),
                )
            o_t = opool.tile([BLK, COLS // 2], bf16, tag=f"o0{h}")
            nc.vector.tensor_scalar_add(o_t[:BLK, :], ps[:BLK, :], b_t[:BLK, :])
            nc.sync.dma_start(o_d[0:BLK, h * (COLS // 2): (h + 1) * (COLS // 2)],
                              o_t[:BLK, :])

        for b in range(1, NBLK):
            r0 = b * BLK
            m = min(BLK, OH - r0)
            k = m + KH - 1
            x_t = xpool.tile([128, INC], bf16)
            nc.sync.dma_start(x_t[:k, :], x_d[r0:r0 + k, :])
            # last block: chunk the free dim so the tail drain+store is short
            nh = 2 if b == NBLK - 1 else 1
            nw = COLS // nh
            for h in range(nh):
                if nh == 1:
                    ps = pspool.tile([BLK, nw], f32, tag="ps1")
                else:
                    ps = pslast.tile([BLK, nw], f32, tag=f"pl{2 + h}")
                for kw in range(KW):
                    nc.tensor.matmul(
                        ps[:m, :],
                        wt_t[:k, kw * BLK: kw * BLK + m],
                        x_t[:k, h * nw + kw: h * nw + kw + nw],
                        start=(kw == 0),
                        stop=(kw == KW - 1),
                    )
                o_t = opool.tile([BLK, nw], bf16, tag=f"o{nh}{h}")
                nc.vector.tensor_scalar_add(o_t[:m, :], ps[:m, :], b_t[:m, :])
                nc.sync.dma_start(o_d[r0:r0 + m, h * nw: (h + 1) * nw],
                                  o_t[:m, :])

    nc.compile()
    return nc


def _toeplitz(weight):
    wtoep = np.zeros((128, KW * BLK), np.float32)
    idx = np.arange(BLK)
    for kw in range(KW):
        for d in range(KH):  # d = h_in - h_out
            wtoep[idx + d, kw * BLK + idx] = weight[d, kw]
    return wtoep


def _prepare_in_maps(x, weight, bias):
    import ml_dtypes
    x = np.asarray(x, dtype=np.float32)
    weight = np.asarray(weight, dtype=np.float32)
    bias = np.asarray(bias, dtype=np.float32)

    x_pad = np.zeros((H, NCORES * COLS + KW - 1), np.float32)
    x_pad[:, :W] = x
    x_bf = x_pad.astype(ml_dtypes.bfloat16)
    wtoep = _toeplitz(weight).astype(ml_dtypes.bfloat16)
    bias_b = np.full((128, 1), bias.reshape(-1)[0], np.float32)

    in_maps = []
    for c in range(NCORES):
        shard = np.ascontiguousarray(x_bf[:, c * COLS: c * COLS + INC])
        in_maps.append({"x": shard, "wt": wtoep, "bias": bias_b})
    return in_maps


def _run(x, weight, bias, trace=False):
    from concourse.bass_utils import run_bass_kernel_spmd

    if "nc" not in _CACHE:
        _CACHE["nc"] = _build_program()
    nc = _CACHE["nc"]

    in_maps = _prepare_in_maps(x, weight, bias)
    res = run_bass_kernel_spmd(nc, in_maps, core_ids=list(range(NCORES)),
                               trace=trace)
    out = np.empty((OH, NCORES * COLS), np.float32)
    for c in range(NCORES):
        out[:, c * COLS: (c + 1) * COLS] = np.asarray(
            res.results[c]["out"], dtype=np.float32)
    return out[:, :OW], res


def kernel(x, weight, bias):
    out, _ = _run(x, weight, bias, trace=False)
    return out

